# revision 4
# baseline (speedup 1.0000x reference)
"""Balanced supervised contrastive regression loss on 8 trn2 cores.

Math: rows of `projections` are unit-norm, so rowmax(logits) = 1/T and
E = exp(s/T - 1/T) + 1e-5 with s = P@P.T. tw_k = weights[l_k] depends only on
the LABEL of k, so every reduction the loss needs is linear in the 121
one-hot row-sums G[u,i] = sum_k 1[l_k=u] * e^{s_ki/T}:
  Q[i,u]    = w[u] * G[u,i]              (label-grouped denominator mass)
  rsE[i]    = sum_u w[u] * G[u,i]        (tw-weighted row sum)
  S1[i]     = sum_u G[u,i]               (plain row sum, for sum_k log E')
and sum_k log(E + 1e-5) ~= (N-1)ln 1e-5 + 1e5*sum_k E (linear log1p - every
off-diagonal 1e5*E is O(0.1)). The diagonal breaks the linearization, so each
core's own 256x256 block (2 of its 16 k-chunks) is handled EXACTLY on the
host in f64 while the device computes G over its 14 non-local chunks only.

Device per core: fp8 DoubleRow logits chain (PSUM fp32), one ACT Exp pass
writing fp8 et in DoubleRow rhs layout, and 7 fp8 DoubleRow one-hot matmuls
(owt is pure 0/1 - exact in fp8; tw applied on host). Ships one [121, 256]
bf16 tile. Host: local blocks + label-space assembly with prefix-sum gathers.
"""
import numpy as np

N, D, VOCAB, OFF = 2048, 512, 121, 40
TEMP = 0.07
NCORES = 8
R = N // NCORES     # 256 anchor columns per core
KC = N // 128       # 16 chunks of 128 k-rows
KCD = KC - 2        # 14 non-local chunks on device
NP = KCD // 2       # 7 DoubleRow pairs
AW = VOCAB + KCD    # aux: [iota(121) | lbl per chunk]
PSCALE = 16.0       # fp8: prescale P into e4m3's sweet spot
EFLOOR = float(np.exp(-1.0 / TEMP))
# exp instr groups over pairs: (pair0, npairs). Balance: small groups early
# (pipeline spin-up + PE p-state ramp), larger later.
EGROUPS = [(0, 1), (1, 1), (2, 1), (3, 2), (5, 2)]
# ptb (pairs 1..6) DMA split: pair1 alone so it lands before PE needs it
PT_DMAS = [(0, 1), (1, 2), (3, 3)]

LAST_EXEC_NS = None
LAST_RESULTS = None


def _build_nc():
    import concourse.bass as bass
    import concourse.mybir as mybir
    from concourse import tile

    import bass_rust as _bass_rust

    f32 = mybir.dt.float32
    bf16 = mybir.dt.bfloat16
    fp8 = mybir.dt.float8e4
    AF = mybir.ActivationFunctionType
    Alu = mybir.AluOpType
    nc = bass.Bass()

    # p0 = per-core ptr rhs (half 0) + pair0 lhsT (half 1): one DMA on the
    # critical startup path. Layout [p, half, A, B, C, D]:
    #   half 0: ptr [ds, s, ihi, ilo];  half 1: [chunk, ds, s, k]
    p0_d = nc.declare_dram_parameter("p0", [128, 2 * 2 * 2 * 2 * 128], fp8, isOutput=False)
    ptb_d = nc.declare_dram_parameter("ptb", [128, 6, 2 * 2 * 2 * 128], fp8, isOutput=False)
    aux_d = nc.declare_dram_parameter("aux", [128, AW], f32, isOutput=False)
    gout_d = nc.declare_dram_parameter("gout", [VOCAB, R], bf16, isOutput=True)

    pm = mybir.MatmulPerfMode.DoubleRow
    act_scale = 1.0 / (TEMP * PSCALE * PSCALE)

    with tile.TileContext(nc) as tc:
        with (
            tc.tile_pool(name="sb", bufs=1) as cpool,
            tc.tile_pool(name="ps", bufs=1, space="PSUM") as pspool,
        ):
            p0_t = cpool.tile([128, 2, 2, 2, 2, 128], fp8, tag="p0")
            nc.sync.dma_start(p0_t[:], p0_d[:])
            aux_t = cpool.tile([128, AW], f32, tag="aux")
            nc.sync.dma_start(aux_t[:], aux_d[:])
            ptb_t = cpool.tile([128, 6, 2, 2, 2, 128], fp8, tag="ptb")
            for j0, nj in PT_DMAS:
                nc.sync.dma_start(ptb_t[:, j0:j0 + nj], ptb_d[:, j0:j0 + nj])

            # pure 0/1 one-hot lhsT blocks, built on the idle DVE from labels.
            # Padded to 128 wide: DoubleRow Ldweights requires full tiles.
            owt_t = cpool.tile([128, NP, 2, 128], fp8, tag="owt")
            nc.vector.memset(owt_t[:, :, :, VOCAB:128], 0.0)
            for j in range(KCD):
                lblap = aux_t[:, VOCAB + j:VOCAB + j + 1]
                nc.vector.tensor_scalar(
                    owt_t[:, j // 2, j % 2, 0:VOCAB], aux_t[:, 0:VOCAB],
                    lblap, None, Alu.is_equal,
                )

            et_t = cpool.tile([128, NP, 2, R], fp8, tag="et")
            gacc = pspool.tile([128, R], f32, tag="gacc")

            def gacc_mms(g0, ng):
                for g in range(g0, g0 + ng):
                    nc.tensor.matmul(gacc[:], owt_t[:, g], et_t[:, g],
                                     start=(g == 0), stop=(g == NP - 1),
                                     perf_mode=pm)

            # two-deep software pipeline: gacc(prev group) queues on PE after
            # logits(cur group) so PE never idles behind a not-yet-ready et
            prev = None
            for g0, ng in EGROUPS:
                lt = pspool.tile([128, ng, 2, R], f32, tag=f"lt{ng}",
                                 bufs=3 if ng == 1 else 2)
                for g in range(g0, g0 + ng):
                    for h in range(2):
                        for ds in range(2):
                            if g == 0:
                                lhsT = p0_t[:, 1, h, ds]
                            else:
                                lhsT = ptb_t[:, g - 1, h, ds]
                            nc.tensor.matmul(
                                lt[:, g - g0, h, :], lhsT, p0_t[:, 0, ds],
                                start=(ds == 0), stop=(ds == 1),
                                perf_mode=pm,
                            )
                nc.scalar.activation(et_t[:, g0:g0 + ng], lt[:], AF.Exp,
                                     bias=0.0, scale=act_scale)
                if prev is not None:
                    gacc_mms(*prev)
                prev = (g0, ng)
            gacc_mms(*prev)

            gs = cpool.tile([VOCAB, R], bf16, tag="gs")
            nc.scalar.copy(gs[:], gacc[0:VOCAB, :])
            nc.sync.dma_start(gout_d[:], gs[:])
    # hardware allows at most one sync wait per instruction (two on
    # InstEventSemaphore): legalize multi-wait instructions before walrus
    _bass_rust.move_matmul_waits_to_ldweights(nc.m)
    _bass_rust.generate_event_semaphores(nc)
    return nc


def _chunks_for_core(c):
    return [j for j in range(KC) if j // 2 != c]


def _prep_inputs(P, lbl):
    """Host-side packing of the SPMD input maps (all per-core)."""
    from concourse.mybir import dt as _dt
    np_fp8 = _dt.np(_dt.float8e4)

    Ps = (P * PSCALE).astype(np_fp8)
    # per-chunk lhsT: pt[p, kc, ds, s, k] = Ps[kc*128 + k, (ds*2+s)*128 + p]
    ptall = np.ascontiguousarray(
        Ps.reshape(KC, 128, 2, 2, 128).transpose(4, 0, 2, 3, 1)
    )  # [128, KC, 2, 2, 128]

    in_maps = []
    for c in range(NCORES):
        ch = _chunks_for_core(c)
        Pc = Ps[c * R:(c + 1) * R]
        ptr = np.ascontiguousarray(
            Pc.reshape(R, 2, 2, 128).transpose(3, 1, 2, 0)
        ).reshape(128, 2 * 2 * R)
        pair0 = ptall[:, ch[0:2]].reshape(128, 2 * 512)
        p0 = np.concatenate([ptr, pair0], 1)
        ptb = np.ascontiguousarray(ptall[:, ch[2:]]).reshape(128, 6, 1024)
        aux = np.zeros((128, AW), np.float32)
        aux[:, :VOCAB] = np.arange(VOCAB, dtype=np.float32)[None, :]
        for j, kc in enumerate(ch):
            aux[:, VOCAB + j] = lbl[kc * 128:(kc + 1) * 128].astype(np.float32)
        in_maps.append({"p0": p0, "ptb": ptb, "aux": aux})
    return in_maps


def _device_run(P, lbl):
    from concourse.bass_utils import run_bass_kernel_spmd

    nc = _build_nc()
    in_maps = _prep_inputs(P, lbl)
    br = run_bass_kernel_spmd(nc, in_maps, list(range(NCORES)))
    global LAST_EXEC_NS, LAST_RESULTS
    LAST_RESULTS = br
    LAST_EXEC_NS = br.exec_time_ns
    # G_rest[u, i] = sum_{k non-local} 1[l_k=u] e^{s_ki/T}
    return np.concatenate(
        [np.asarray(r["gout"]).astype(np.float64) for r in br.results], 1)


def _host_g_rest(P, lbl):
    """Fallback: emulate the device G_rest on host (fp64, no quantization)."""
    G = np.zeros((VOCAB, N))
    for c in range(NCORES):
        ii = slice(c * R, (c + 1) * R)
        mask = np.ones(N, bool)
        mask[ii] = False
        s = P[mask].astype(np.float64) @ P[ii].astype(np.float64).T
        et = np.exp(s / TEMP)
        np.add.at(G[:, ii], lbl[mask], et)
    return G


def _assemble(G_rest, P, lbl, w):
    """Label-space assembly; local 256x256 blocks handled exactly in f64."""
    tw = w[lbl]
    TWS = tw.sum()

    # local blocks: exact E, exact log1p (incl. the troublesome diagonal)
    G_full = EFLOOR * G_rest           # -> sum of E over non-local k
    L_local = np.zeros(N)
    for c in range(NCORES):
        ii = slice(c * R, (c + 1) * R)
        Pl = P[ii].astype(np.float64)
        E_loc = np.exp((Pl @ Pl.T - 1.0) / TEMP)
        L_local[ii] = np.log1p(1e5 * E_loc).sum(0)
        np.add.at(G_full[:, ii], lbl[ii], E_loc)

    S1_rest = G_rest.sum(0)
    slacc = L_local + 1e5 * EFLOOR * S1_rest   # sum_k log1p(1e5 E)
    sumlogE = slacc - N * np.log(1e5)

    Q = (w[:, None] * G_full).T                       # [N, 121]
    rsE = (w[None, :] @ G_full).ravel() + 1e-5 * TWS  # [N]

    cw = np.bincount(lbl, weights=tw, minlength=VOCAB)
    W = Q + 1e-5 * cw[None, :]
    PS1 = np.concatenate([np.zeros((N, 1)), np.cumsum(W, 1)], 1)  # [N,122]

    vcol = np.arange(VOCAB)[:, None]
    B = np.abs(vcol - lbl[None, :])                  # [121, N]
    lo = np.clip(vcol - B + 1, 0, VOCAB)
    hi1 = np.clip(vcol + B, 0, VOCAB)
    jj = np.arange(N)[None, :]
    inner = PS1[jj, hi1] - PS1[jj, lo]
    inner[B == 0] = 0.0
    Dv = rsE[None, :] - inner                        # [121, N]
    ltw = np.log(tw)
    SLT = ltw.sum()
    LDsum = SLT + np.log(Dv).sum(1)                  # [121]

    rowsumA = tw * rsE
    rowsumLA = sumlogE + N * ltw + SLT
    LAdiag = np.log1p(1e-5) + 2.0 * ltw
    per = (LDsum[lbl] - np.log(rowsumA) - (rowsumLA - LAdiag)) / (N - 1 + 1e-5)
    return per.mean()


def kernel(projections, targets, weights):
    P = np.asarray(projections, np.float32)
    t = np.asarray(targets).astype(np.int64)
    w = np.asarray(weights, np.float64)
    lbl = (t - OFF).astype(np.int64)

    try:
        G_rest = _device_run(P, lbl)
    except Exception as e:  # pragma: no cover - safety net
        import traceback
        traceback.print_exc()
        print("DEVICE PATH FAILED - host fallback:", e)
        G_rest = _host_g_rest(P, lbl)

    return np.float32(_assemble(G_rest, P, lbl, w))


# revision 19
# speedup vs baseline: 1.1212x; 1.1212x over previous
"""Balanced supervised contrastive regression loss on 8 trn2 cores.

Math: rows of `projections` are unit-norm, so rowmax(logits) = 1/T and
E = exp(s/T - 1/T) + 1e-5 with s = P@P.T. tw_k = weights[l_k] depends only on
the LABEL of k, so every reduction the loss needs is linear in the 121
one-hot row-sums G[u,i] = sum_k 1[l_k=u] * e^{s_ki/T}:
  Q[i,u]    = w[u] * G[u,i]              (label-grouped denominator mass)
  rsE[i]    = sum_u w[u] * G[u,i]        (tw-weighted row sum)
  S1[i]     = sum_u G[u,i]               (plain row sum, for sum_k log E')
and sum_k log(E + 1e-5) ~= (N-1)ln 1e-5 + 1e5*sum_k E (linear log1p - every
off-diagonal 1e5*E is O(0.1)). The diagonal breaks the linearization, so each
core's own 256x256 block (2 of its 16 k-chunks) is handled EXACTLY on the
host in f64 while the device computes G over its 14 non-local chunks only.

Device per core: fp8 DoubleRow logits chain (PSUM fp32), one ACT Exp pass
writing fp8 et in DoubleRow rhs layout, and 7 fp8 DoubleRow one-hot matmuls
(owt is pure 0/1 - exact in fp8; tw applied on host). Ships one [121, 256]
bf16 tile. Host: local blocks + label-space assembly with prefix-sum gathers.
"""
import numpy as np

N, D, VOCAB, OFF = 2048, 512, 121, 40
TEMP = 0.07
NCORES = 8
R = N // NCORES     # 256 anchor columns per core
KC = N // 128       # 16 chunks of 128 k-rows
KCD = KC - 2        # 14 non-local chunks on device
NP = KCD // 2       # 7 DoubleRow pairs
AW = VOCAB + KCD    # aux: [iota(121) | lbl per chunk]
PSCALE = 16.0       # fp8: prescale P into e4m3's sweet spot
EFLOOR = float(np.exp(-1.0 / TEMP))
# exp instr groups over pairs: (pair0, npairs). Balance: small groups early
# (pipeline spin-up + PE p-state ramp), larger later.
EGROUPS = [(0, 1), (1, 1), (2, 1), (3, 2), (5, 2)]
# ptb (pairs 1..6) DMA split: early pairs first so each lands just before
# PE needs it (transfers serialize on the DMA engines)
PT_DMAS = [(0, 2), (2, 2), (4, 2)]

LAST_EXEC_NS = None
LAST_RESULTS = None


def _build_nc():
    import concourse.bass as bass
    import concourse.mybir as mybir
    from concourse import tile

    import bass_rust as _bass_rust

    f32 = mybir.dt.float32
    bf16 = mybir.dt.bfloat16
    fp8 = mybir.dt.float8e4
    i16 = mybir.dt.int16
    AF = mybir.ActivationFunctionType
    Alu = mybir.AluOpType
    nc = bass.Bass()

    # p0 = per-core ptr rhs (half 0) + pair0 lhsT (half 1): one DMA on the
    # critical startup path. Layout [p, half, A, B, C, D]:
    #   half 0: ptr [ds, s, ihi, ilo];  half 1: [chunk, ds, s, k]
    p0_d = nc.declare_dram_parameter("p0", [128, 2 * 2 * 2 * 2 * 128], fp8, isOutput=False)
    ptb_d = nc.declare_dram_parameter("ptb", [128, 6, 2 * 2 * 2 * 128], fp8, isOutput=False)
    aux_d = nc.declare_dram_parameter("aux", [128, AW], f32, isOutput=False)
    gout_d = nc.declare_dram_parameter("gout", [128, R], bf16, isOutput=True)

    pm = mybir.MatmulPerfMode.DoubleRow
    act_scale = 1.0 / (TEMP * PSCALE * PSCALE)

    with tile.TileContext(nc) as tc:
        with (
            tc.tile_pool(name="sb", bufs=1) as cpool,
            tc.tile_pool(name="ps", bufs=1, space="PSUM") as pspool,
        ):
            p0_t = cpool.tile([128, 2, 2, 2, 2, 128], fp8, tag="p0")
            nc.sync.dma_start(p0_t[:], p0_d[:])
            ptb_t = cpool.tile([128, 6, 2, 2, 2, 128], fp8, tag="ptb")
            aux_t = cpool.tile([128, AW], f32, tag="aux")
            pt_dmas = list(PT_DMAS)
            j0, nj = pt_dmas.pop(0)
            nc.sync.dma_start(ptb_t[:, j0:j0 + nj], ptb_d[:, j0:j0 + nj])
            nc.sync.dma_start(aux_t[:], aux_d[:])
            for j0, nj in pt_dmas:
                nc.sync.dma_start(ptb_t[:, j0:j0 + nj], ptb_d[:, j0:j0 + nj])

            # identity scatter indices (idx[p, s] = 16*s + p) for the gout
            # writeback; descriptors pre-generate on the idle Pool engine and
            # the trailing trigger_dma fires them once gs is written, skipping
            # the HWDGE-gen + DGE-delay latency on the critical output path.
            sidx_t = cpool.tile([128, 8], i16, tag="sidx")
            nc.gpsimd.iota(sidx_t[:], [[16, 8]], base=0, channel_multiplier=1)
            # completion must tick the tile scope's DMASW0 lane sem (the
            # end-of-kernel drain waits on it by name)
            kvsem = tc.sems.swdge_block()[0]

            # pure 0/1 one-hot lhsT blocks, built on the idle DVE from labels.
            # Padded to 128 wide: DoubleRow Ldweights requires full tiles.
            owt_t = cpool.tile([128, NP, 2, 128], fp8, tag="owt")
            nc.vector.memset(owt_t[:, :, :, VOCAB:128], 0.0)
            for j in range(KCD):
                lblap = aux_t[:, VOCAB + j:VOCAB + j + 1]
                nc.vector.tensor_scalar(
                    owt_t[:, j // 2, j % 2, 0:VOCAB], aux_t[:, 0:VOCAB],
                    lblap, None, Alu.is_equal,
                )

            et_t = cpool.tile([128, NP, 2, R], fp8, tag="et")
            gacc = pspool.tile([128, R], f32, tag="gacc")
            gs = cpool.tile([128, 1, R], bf16, tag="gs")

            def gacc_mms(g0, ng):
                for g in range(g0, g0 + ng):
                    nc.tensor.matmul(gacc[:], owt_t[:, g], et_t[:, g],
                                     start=(g == 0), stop=(g == NP - 1),
                                     perf_mode=pm)

            # two-deep software pipeline: gacc(prev group) queues on PE after
            # logits(cur group) so PE never idles behind a not-yet-ready et
            prev = None
            for g0, ng in EGROUPS:
                lt = pspool.tile([128, ng, 2, R], f32, tag=f"lt{ng}",
                                 bufs=3 if ng == 1 else 2)
                for g in range(g0, g0 + ng):
                    for h in range(2):
                        for ds in range(2):
                            if g == 0:
                                lhsT = p0_t[:, 1, h, ds]
                            else:
                                lhsT = ptb_t[:, g - 1, h, ds]
                            nc.tensor.matmul(
                                lt[:, g - g0, h, :], lhsT, p0_t[:, 0, ds],
                                start=(ds == 0), stop=(ds == 1),
                                perf_mode=pm,
                            )
                nc.scalar.activation(et_t[:, g0:g0 + ng], lt[:], AF.Exp,
                                     bias=0.0, scale=act_scale)
                if prev is not None:
                    gacc_mms(*prev)
                prev = (g0, ng)
            gacc_mms(*prev)

            nc.scalar.copy(gs[:, 0, :], gacc[:])
            # scatter-add with identity indices == plain write (both SPMD
            # execution paths pre-zero ExternalOutput buffers). The prep's
            # read of gs is deferred to the trigger, so the prep itself runs
            # early on the idle Pool engine.
            nc.gpsimd.dma_scatter_add(gout_d[:], gs[:], sidx_t[:], 128, 128, R,
                                      prepare_only=True, sem=kvsem)
            nc.gpsimd.trigger_dma(count=None)
    # hardware allows at most one sync wait per instruction (two on
    # InstEventSemaphore): legalize multi-wait instructions before walrus
    _bass_rust.move_matmul_waits_to_ldweights(nc.m)
    _bass_rust.generate_event_semaphores(nc)
    return nc


def _chunks_for_core(c):
    return [j for j in range(KC) if j // 2 != c]


def _prep_inputs(P, lbl):
    """Host-side packing of the SPMD input maps (all per-core)."""
    from concourse.mybir import dt as _dt
    np_fp8 = _dt.np(_dt.float8e4)

    Ps = (P * PSCALE).astype(np_fp8)
    # per-chunk lhsT: pt[p, kc, ds, s, k] = Ps[kc*128 + k, (ds*2+s)*128 + p]
    ptall = np.ascontiguousarray(
        Ps.reshape(KC, 128, 2, 2, 128).transpose(4, 0, 2, 3, 1)
    )  # [128, KC, 2, 2, 128]

    in_maps = []
    for c in range(NCORES):
        ch = _chunks_for_core(c)
        Pc = Ps[c * R:(c + 1) * R]
        ptr = np.ascontiguousarray(
            Pc.reshape(R, 2, 2, 128).transpose(3, 1, 2, 0)
        ).reshape(128, 2 * 2 * R)
        pair0 = ptall[:, ch[0:2]].reshape(128, 2 * 512)
        p0 = np.concatenate([ptr, pair0], 1)
        ptb = np.ascontiguousarray(ptall[:, ch[2:]]).reshape(128, 6, 1024)
        aux = np.zeros((128, AW), np.float32)
        aux[:, :VOCAB] = np.arange(VOCAB, dtype=np.float32)[None, :]
        for j, kc in enumerate(ch):
            aux[:, VOCAB + j] = lbl[kc * 128:(kc + 1) * 128].astype(np.float32)
        in_maps.append({"p0": p0, "ptb": ptb, "aux": aux})
    return in_maps


def _device_run(P, lbl):
    from concourse.bass_utils import run_bass_kernel_spmd

    nc = _build_nc()
    in_maps = _prep_inputs(P, lbl)
    br = run_bass_kernel_spmd(nc, in_maps, list(range(NCORES)))
    global LAST_EXEC_NS, LAST_RESULTS
    LAST_RESULTS = br
    LAST_EXEC_NS = br.exec_time_ns
    # G_rest[u, i] = sum_{k non-local} 1[l_k=u] e^{s_ki/T}
    return np.concatenate(
        [np.asarray(r["gout"])[:VOCAB].astype(np.float64)
         for r in br.results], 1)


def _host_g_rest(P, lbl):
    """Fallback: emulate the device G_rest on host (fp64, no quantization)."""
    G = np.zeros((VOCAB, N))
    for c in range(NCORES):
        ii = slice(c * R, (c + 1) * R)
        mask = np.ones(N, bool)
        mask[ii] = False
        s = P[mask].astype(np.float64) @ P[ii].astype(np.float64).T
        et = np.exp(s / TEMP)
        np.add.at(G[:, ii], lbl[mask], et)
    return G


def _assemble(G_rest, P, lbl, w):
    """Label-space assembly; local 256x256 blocks handled exactly in f64."""
    tw = w[lbl]
    TWS = tw.sum()

    # local blocks: exact E, exact log1p (incl. the troublesome diagonal)
    G_full = EFLOOR * G_rest           # -> sum of E over non-local k
    L_local = np.zeros(N)
    for c in range(NCORES):
        ii = slice(c * R, (c + 1) * R)
        Pl = P[ii].astype(np.float64)
        E_loc = np.exp((Pl @ Pl.T - 1.0) / TEMP)
        L_local[ii] = np.log1p(1e5 * E_loc).sum(0)
        np.add.at(G_full[:, ii], lbl[ii], E_loc)

    S1_rest = G_rest.sum(0)
    slacc = L_local + 1e5 * EFLOOR * S1_rest   # sum_k log1p(1e5 E)
    sumlogE = slacc - N * np.log(1e5)

    Q = (w[:, None] * G_full).T                       # [N, 121]
    rsE = (w[None, :] @ G_full).ravel() + 1e-5 * TWS  # [N]

    cw = np.bincount(lbl, weights=tw, minlength=VOCAB)
    W = Q + 1e-5 * cw[None, :]
    PS1 = np.concatenate([np.zeros((N, 1)), np.cumsum(W, 1)], 1)  # [N,122]

    vcol = np.arange(VOCAB)[:, None]
    B = np.abs(vcol - lbl[None, :])                  # [121, N]
    lo = np.clip(vcol - B + 1, 0, VOCAB)
    hi1 = np.clip(vcol + B, 0, VOCAB)
    jj = np.arange(N)[None, :]
    inner = PS1[jj, hi1] - PS1[jj, lo]
    inner[B == 0] = 0.0
    Dv = rsE[None, :] - inner                        # [121, N]
    ltw = np.log(tw)
    SLT = ltw.sum()
    LDsum = SLT + np.log(Dv).sum(1)                  # [121]

    rowsumA = tw * rsE
    rowsumLA = sumlogE + N * ltw + SLT
    LAdiag = np.log1p(1e-5) + 2.0 * ltw
    per = (LDsum[lbl] - np.log(rowsumA) - (rowsumLA - LAdiag)) / (N - 1 + 1e-5)
    return per.mean()


def kernel(projections, targets, weights):
    P = np.asarray(projections, np.float32)
    t = np.asarray(targets).astype(np.int64)
    w = np.asarray(weights, np.float64)
    lbl = (t - OFF).astype(np.int64)

    try:
        G_rest = _device_run(P, lbl)
    except Exception as e:  # pragma: no cover - safety net
        import traceback
        traceback.print_exc()
        print("DEVICE PATH FAILED - host fallback:", e)
        G_rest = _host_g_rest(P, lbl)

    return np.float32(_assemble(G_rest, P, lbl, w))
